# revision 4
# baseline (speedup 1.0000x reference)
"""Hard-batch-mining triplet loss on 8 Trainium2 NeuronCores.

Reference computation (B=4096, D=2048, NCLS=64, margin=0.3):
    sq = rownorm2(X);  dist = sqrt(clip(sq_i + sq_j - 2 X X^T, 1e-12))
    d_pos = max_{same class} dist;  d_neg = min_{diff class} dist
    loss = mean(relu(d_pos - d_neg + margin))

Distribution: row-parallel. Core c owns 512 rows and computes its
[512, 4096] block of the Gram matrix against all columns.

The whole masked-distance epilogue is folded into the GEMM by augmenting
the contraction dimension:
    rhs  rows: [ X^T (bf16) ; BIG*onehot(class_j) ; hi(sq_j-2048) ; lo ]
    lhsT rows: [ -2*X_blk^T (bf16) ; onehot(class_i) ; 1 ; 1 ]
so PSUM accumulates  p[i,j] = -2*G[i,j] + (sq_j-2048) + BIG*[same class]
in fp32.  Since sqrt/clip are monotone, the masked row reductions reduce
p directly:
    max_j p = BIG + max_same(sq_j - 2048 - 2G)   (BIG dominates)
    min_j p =       min_diff(sq_j - 2048 - 2G)
Per-row epilogue (on [128,1] vectors): add sq_i + 2048 (-BIG), clip,
sqrt, subtract, relu.  Host sums the 4096 per-row losses.
"""

import sys

import numpy as np

_B, _D, _NCLS = 4096, 2048, 64
_NCORES = 8
_MB = _B // _NCORES  # 512 rows per core
_P = 128
_MT = _MB // _P  # 4 m-tiles per core
_NF = 512  # matmul free dim / PSUM bank
_NCH = _B // _NF  # 8 n-chunks
_KT = 17  # 16 k-tiles of X + 1 augmented tile (66 used rows, zero-padded)
_KROWS = _KT * _P
_BIG = 65536.0
_C = 2048.0  # centering constant for sq rows
_MARGIN = 0.3


def _import_concourse():
    try:
        import concourse.bass  # noqa: F401
    except ImportError:
        for p in ("/opt/trn_rl_repo", "/root/.axon_site/_ro/trn_rl_repo"):
            if p not in sys.path:
                sys.path.insert(0, p)
        import concourse.bass  # noqa: F401


def build_nc(finalize=True):
    """Build the single-core Bass/Tile program (SPMD: same program, per-core data)."""
    _import_concourse()
    import concourse.bacc as bacc
    import concourse.mybir as mybir
    import concourse.tile as tile

    f32 = mybir.dt.float32
    bf16 = mybir.dt.bfloat16

    nc = bacc.Bacc(
        "TRN2",
        target_bir_lowering=False,
        debug=False,
        num_devices=_NCORES,
    )

    rhs_d = nc.declare_dram_parameter("rhs", [_NCH, _P, _KT, _NF], bf16, isOutput=False)
    lhs_d = nc.declare_dram_parameter("lhs", [_P, _KT, _MT, _P], bf16, isOutput=False)
    srow_d = nc.declare_dram_parameter("srow", [_P, 2, _MT], f32, isOutput=False)
    loss_d = nc.declare_dram_parameter("losses", [_P, _MT], f32, isOutput=True)

    with tile.TileContext(nc) as tc:
        with (
            tc.tile_pool(name="persist", bufs=1) as persist,
            tc.tile_pool(name="stream", bufs=3) as stream,
            tc.tile_pool(name="ps", bufs=2, space="PSUM") as psp,
            tc.tile_pool(name="small", bufs=1) as small,
        ):
            lhs_t = persist.tile([_P, _KT, _MT, _P], bf16, name="lhs_t")
            nc.sync.dma_start(out=lhs_t[:], in_=lhs_d[:])
            srow_t = persist.tile([_P, 2, _MT], f32, name="srow_t")
            nc.sync.dma_start(out=srow_t[:], in_=srow_d[:])

            maxacc = persist.tile([_P, _MT, _NCH], f32, name="maxacc")
            minacc = persist.tile([_P, _MT, _NCH], f32, name="minacc")
            loss_t = small.tile([_P, _MT], f32, name="loss_t")

            for n in range(_NCH):
                chunk = stream.tile([_P, _KT, _NF], bf16, name="chunk", tag="chunk")
                nc.sync.dma_start(out=chunk[:], in_=rhs_d[n])
                for m in range(_MT):
                    ps = psp.tile([_P, _NF], f32, name="ps", tag=f"ps{m}")
                    for kt in range(_KT):
                        nc.tensor.matmul(
                            ps[:],
                            lhs_t[:, kt, m, :],
                            chunk[:, kt, :],
                            start=(kt == 0),
                            stop=(kt == _KT - 1),
                        )
                    nc.vector.tensor_reduce(
                        maxacc[:, m, n : n + 1],
                        ps[:],
                        axis=mybir.AxisListType.X,
                        op=mybir.AluOpType.max,
                    )
                    nc.vector.tensor_reduce(
                        minacc[:, m, n : n + 1],
                        ps[:],
                        axis=mybir.AxisListType.X,
                        op=mybir.AluOpType.min,
                    )

            for m in range(_MT):
                mx = small.tile([_P, 1], f32, name=f"mx{m}")
                mn = small.tile([_P, 1], f32, name=f"mn{m}")
                nc.vector.tensor_reduce(
                    mx[:], maxacc[:, m, :], axis=mybir.AxisListType.X, op=mybir.AluOpType.max
                )
                nc.vector.tensor_reduce(
                    mn[:], minacc[:, m, :], axis=mybir.AxisListType.X, op=mybir.AluOpType.min
                )
                d2p = small.tile([_P, 1], f32, name=f"d2p{m}")
                d2n = small.tile([_P, 1], f32, name=f"d2n{m}")
                # d2_pos = max(mx + (sq_i + C - BIG), 1e-12)
                nc.vector.tensor_scalar(
                    d2p[:],
                    mx[:],
                    srow_t[:, 0, m : m + 1],
                    1e-12,
                    op0=mybir.AluOpType.add,
                    op1=mybir.AluOpType.max,
                )
                # d2_neg = max(mn + (sq_i + C), 1e-12)
                nc.vector.tensor_scalar(
                    d2n[:],
                    mn[:],
                    srow_t[:, 1, m : m + 1],
                    1e-12,
                    op0=mybir.AluOpType.add,
                    op1=mybir.AluOpType.max,
                )
                dp = small.tile([_P, 1], f32, name=f"dp{m}")
                dn = small.tile([_P, 1], f32, name=f"dn{m}")
                nc.scalar.sqrt(dp[:], d2p[:])
                nc.scalar.sqrt(dn[:], d2n[:])
                diff = small.tile([_P, 1], f32, name=f"diff{m}")
                nc.vector.tensor_tensor(
                    diff[:], dp[:], dn[:], op=mybir.AluOpType.subtract
                )
                # loss = max(diff + margin, 0)
                nc.vector.tensor_scalar(
                    loss_t[:, m : m + 1],
                    diff[:],
                    _MARGIN,
                    0.0,
                    op0=mybir.AluOpType.add,
                    op1=mybir.AluOpType.max,
                )

            nc.sync.dma_start(out=loss_d[:], in_=loss_t[:])

    if finalize:
        nc.finalize()
    return nc


def prep_inputs(inputs, targets):
    """Host-side shard prep. Returns per-core input maps."""
    import ml_dtypes

    X = np.asarray(inputs, dtype=np.float32)
    t = np.asarray(targets).astype(np.int64)
    assert X.shape == (_B, _D) and t.shape == (_B,)

    bf = ml_dtypes.bfloat16
    XTb = np.ascontiguousarray(X.T).astype(bf)  # [D, B] bf16
    sq = np.einsum("ij,ij->i", X.astype(np.float64), X.astype(np.float64)).astype(
        np.float32
    )
    res = sq - np.float32(_C)
    hi = res.astype(bf)
    lo = (res - hi.astype(np.float32)).astype(bf)

    # Shared augmented rhs [KROWS, B] -> DMA layout [NCH, P, KT, NF]
    rhs_aug = np.zeros((_KROWS, _B), dtype=bf)
    rhs_aug[:_D] = XTb
    rhs_aug[_D + t, np.arange(_B)] = bf(_BIG)  # BIG * onehot(class_j)
    rhs_aug[_D + _NCLS] = hi
    rhs_aug[_D + _NCLS + 1] = lo
    rhs_host = np.ascontiguousarray(
        rhs_aug.reshape(_KT, _P, _NCH, _NF).transpose(2, 1, 0, 3)
    )

    in_maps = []
    for c in range(_NCORES):
        blk = slice(c * _MB, (c + 1) * _MB)
        lhs_aug = np.zeros((_KROWS, _MB), dtype=bf)
        lhs_aug[:_D] = (XTb[:, blk].astype(np.float32) * -2.0).astype(bf)  # exact
        tb = t[blk]
        lhs_aug[_D + tb, np.arange(_MB)] = bf(1.0)  # onehot(class_i)
        lhs_aug[_D + _NCLS] = bf(1.0)
        lhs_aug[_D + _NCLS + 1] = bf(1.0)
        lhs_host = np.ascontiguousarray(
            lhs_aug.reshape(_KT, _P, _MT, _P).transpose(1, 0, 2, 3)
        )

        sqb = sq[blk].reshape(_MT, _P)  # [m, p]
        srow = np.empty((_P, 2, _MT), dtype=np.float32)
        srow[:, 0, :] = (sqb + np.float32(_C) - np.float32(_BIG)).T
        srow[:, 1, :] = (sqb + np.float32(_C)).T

        in_maps.append({"rhs": rhs_host, "lhs": lhs_host, "srow": srow})
    return in_maps


def combine_outputs(results):
    """results: list of per-core {'losses': [P, MT]} -> scalar loss."""
    rows = []
    for c in range(_NCORES):
        ls = np.asarray(results[c]["losses"])  # [P, MT]
        rows.append(ls.T.reshape(-1))  # row index m*128+p
    all_rows = np.concatenate(rows)
    return np.asarray(all_rows.mean(dtype=np.float64), dtype=np.float32)


def kernel(inputs, targets):
    _import_concourse()
    from concourse.bass_utils import run_bass_kernel_spmd

    nc = build_nc()
    in_maps = prep_inputs(inputs, targets)
    res = run_bass_kernel_spmd(nc, in_maps, core_ids=list(range(_NCORES)))
    return combine_outputs(res.results)


# revision 7
# speedup vs baseline: 1.0167x; 1.0167x over previous
"""Hard-batch-mining triplet loss on 8 Trainium2 NeuronCores.

Reference computation (B=4096, D=2048, NCLS=64, margin=0.3):
    sq = rownorm2(X);  dist = sqrt(clip(sq_i + sq_j - 2 X X^T, 1e-12))
    d_pos = max_{same class} dist;  d_neg = min_{diff class} dist
    loss = mean(relu(d_pos - d_neg + margin))

Distribution: row-parallel. Core c owns 512 rows and computes its
[512, 4096] block of the Gram matrix against all columns.

The whole masked-distance epilogue is folded into the GEMM by augmenting
the contraction dimension:
    rhs  rows: [ X^T (bf16) ; BIG*onehot(class_j) ; hi(sq_j-2048) ; lo ]
    lhsT rows: [ -2*X_blk^T (bf16) ; onehot(class_i) ; 1 ; 1 ]
so PSUM accumulates  p[i,j] = -2*G[i,j] + (sq_j-2048) + BIG*[same class]
in fp32.  Since sqrt/clip are monotone, the masked row reductions reduce
p directly:
    max_j p = BIG + max_same(sq_j - 2048 - 2G)   (BIG dominates)
    min_j p =       min_diff(sq_j - 2048 - 2G)
Per-row epilogue (on [128,1] vectors): add sq_i + 2048 (-BIG), clip,
sqrt, subtract, relu.  Host sums the 4096 per-row losses.
"""

import sys

import numpy as np

_B, _D, _NCLS = 4096, 2048, 64
_NCORES = 8
_MB = _B // _NCORES  # 512 rows per core
_P = 128
_MT = _MB // _P  # 4 m-tiles per core
_NF = 512  # matmul free dim / PSUM bank
_NCH = _B // _NF  # 8 n-chunks
_KT = 17  # 16 k-tiles of X + 1 augmented tile (66 used rows, zero-padded)
_KROWS = _KT * _P
_BIG = 65536.0
_C = 2048.0  # centering constant for sq rows
_MARGIN = 0.3


def _import_concourse():
    try:
        import concourse.bass  # noqa: F401
    except ImportError:
        for p in ("/opt/trn_rl_repo", "/root/.axon_site/_ro/trn_rl_repo"):
            if p not in sys.path:
                sys.path.insert(0, p)
        import concourse.bass  # noqa: F401


def build_nc(finalize=True):
    """Build the single-core Bass/Tile program (SPMD: same program, per-core data)."""
    _import_concourse()
    import concourse.bacc as bacc
    import concourse.mybir as mybir
    import concourse.tile as tile

    f32 = mybir.dt.float32
    bf16 = mybir.dt.bfloat16

    nc = bacc.Bacc(
        "TRN2",
        target_bir_lowering=False,
        debug=False,
        num_devices=_NCORES,
    )

    rhs_d = nc.declare_dram_parameter("rhs", [_NCH, _P, _KT, _NF], bf16, isOutput=False)
    lhs_d = nc.declare_dram_parameter("lhs", [_P, _KT, _MT, _P], bf16, isOutput=False)
    srow_d = nc.declare_dram_parameter("srow", [_P, 2, _MT], f32, isOutput=False)
    loss_d = nc.declare_dram_parameter("losses", [_P, _MT], f32, isOutput=True)

    with tile.TileContext(nc) as tc:
        with (
            tc.tile_pool(name="persist", bufs=1) as persist,
            tc.tile_pool(name="stream", bufs=4) as stream,
            tc.tile_pool(name="ps", bufs=2, space="PSUM") as psp,
            tc.tile_pool(name="small", bufs=1) as small,
        ):
            lhs_t = persist.tile([_P, _KT, _MT, _P], bf16, name="lhs_t")
            srow_t = persist.tile([_P, 2, _MT], f32, name="srow_t")
            nc.sync.dma_start(out=srow_t[:], in_=srow_d[:])

            maxacc = persist.tile([_P, _MT, _NCH], f32, name="maxacc")
            minacc = persist.tile([_P, _MT, _NCH], f32, name="minacc")
            loss_t = small.tile([_P, _MT], f32, name="loss_t")

            for n in range(_NCH):
                chunk = stream.tile([_P, _KT, _NF], bf16, name="chunk", tag="chunk")
                if n == 0:
                    # Fine-grained pieces so the first matmul starts after
                    # one k-tile of lhs + chunk0 lands, not after 4.4 MB.
                    for kt in range(_KT):
                        nc.sync.dma_start(
                            out=lhs_t[:, kt], in_=lhs_d[:, kt]
                        )
                        nc.sync.dma_start(
                            out=chunk[:, kt], in_=rhs_d[n, :, kt : kt + 1]
                        )
                elif n == 1:
                    h = _KT // 2
                    nc.sync.dma_start(out=chunk[:, :h], in_=rhs_d[n, :, :h])
                    nc.sync.dma_start(out=chunk[:, h:], in_=rhs_d[n, :, h:])
                else:
                    nc.sync.dma_start(out=chunk[:], in_=rhs_d[n])
                for m in range(_MT):
                    ps = psp.tile([_P, _NF], f32, name="ps", tag=f"ps{m}")
                    for kt in range(_KT):
                        nc.tensor.matmul(
                            ps[:],
                            lhs_t[:, kt, m, :],
                            chunk[:, kt, :],
                            start=(kt == 0),
                            stop=(kt == _KT - 1),
                        )
                    nc.vector.tensor_reduce(
                        maxacc[:, m, n : n + 1],
                        ps[:],
                        axis=mybir.AxisListType.X,
                        op=mybir.AluOpType.max,
                    )
                    nc.vector.tensor_reduce(
                        minacc[:, m, n : n + 1],
                        ps[:],
                        axis=mybir.AxisListType.X,
                        op=mybir.AluOpType.min,
                    )

            # Vectorized epilogue over all 4 m-tiles at once ([128, 4] ops).
            mx4 = small.tile([_P, _MT], f32, name="mx4")
            mn4 = small.tile([_P, _MT], f32, name="mn4")
            nc.vector.tensor_reduce(
                mx4[:], maxacc[:], axis=mybir.AxisListType.X, op=mybir.AluOpType.max
            )
            nc.vector.tensor_reduce(
                mn4[:], minacc[:], axis=mybir.AxisListType.X, op=mybir.AluOpType.min
            )
            d2p = small.tile([_P, _MT], f32, name="d2p")
            d2n = small.tile([_P, _MT], f32, name="d2n")
            # d2_pos = max(mx + (sq_i + C - BIG), 1e-12); d2_neg likewise.
            nc.vector.tensor_tensor(
                d2p[:], mx4[:], srow_t[:, 0, :], op=mybir.AluOpType.add
            )
            nc.vector.tensor_tensor(
                d2n[:], mn4[:], srow_t[:, 1, :], op=mybir.AluOpType.add
            )
            nc.vector.tensor_scalar_max(d2p[:], d2p[:], 1e-12)
            nc.vector.tensor_scalar_max(d2n[:], d2n[:], 1e-12)
            dp = small.tile([_P, _MT], f32, name="dp")
            dn = small.tile([_P, _MT], f32, name="dn")
            nc.scalar.sqrt(dp[:], d2p[:])
            nc.scalar.sqrt(dn[:], d2n[:])
            diff = small.tile([_P, _MT], f32, name="diff")
            nc.vector.tensor_tensor(
                diff[:], dp[:], dn[:], op=mybir.AluOpType.subtract
            )
            # loss = max(diff + margin, 0)
            nc.vector.tensor_scalar(
                loss_t[:],
                diff[:],
                _MARGIN,
                0.0,
                op0=mybir.AluOpType.add,
                op1=mybir.AluOpType.max,
            )

            nc.sync.dma_start(out=loss_d[:], in_=loss_t[:])

    if finalize:
        nc.finalize()
    return nc


def prep_inputs(inputs, targets):
    """Host-side shard prep. Returns per-core input maps."""
    import ml_dtypes

    X = np.asarray(inputs, dtype=np.float32)
    t = np.asarray(targets).astype(np.int64)
    assert X.shape == (_B, _D) and t.shape == (_B,)

    bf = ml_dtypes.bfloat16
    XTb = np.ascontiguousarray(X.T).astype(bf)  # [D, B] bf16
    sq = np.einsum("ij,ij->i", X.astype(np.float64), X.astype(np.float64)).astype(
        np.float32
    )
    res = sq - np.float32(_C)
    hi = res.astype(bf)
    lo = (res - hi.astype(np.float32)).astype(bf)

    # Shared augmented rhs [KROWS, B] -> DMA layout [NCH, P, KT, NF]
    rhs_aug = np.zeros((_KROWS, _B), dtype=bf)
    rhs_aug[:_D] = XTb
    rhs_aug[_D + t, np.arange(_B)] = bf(_BIG)  # BIG * onehot(class_j)
    rhs_aug[_D + _NCLS] = hi
    rhs_aug[_D + _NCLS + 1] = lo
    rhs_host = np.ascontiguousarray(
        rhs_aug.reshape(_KT, _P, _NCH, _NF).transpose(2, 1, 0, 3)
    )

    in_maps = []
    for c in range(_NCORES):
        blk = slice(c * _MB, (c + 1) * _MB)
        lhs_aug = np.zeros((_KROWS, _MB), dtype=bf)
        lhs_aug[:_D] = (XTb[:, blk].astype(np.float32) * -2.0).astype(bf)  # exact
        tb = t[blk]
        lhs_aug[_D + tb, np.arange(_MB)] = bf(1.0)  # onehot(class_i)
        lhs_aug[_D + _NCLS] = bf(1.0)
        lhs_aug[_D + _NCLS + 1] = bf(1.0)
        lhs_host = np.ascontiguousarray(
            lhs_aug.reshape(_KT, _P, _MT, _P).transpose(1, 0, 2, 3)
        )

        sqb = sq[blk].reshape(_MT, _P)  # [m, p]
        srow = np.empty((_P, 2, _MT), dtype=np.float32)
        srow[:, 0, :] = (sqb + np.float32(_C) - np.float32(_BIG)).T
        srow[:, 1, :] = (sqb + np.float32(_C)).T

        in_maps.append({"rhs": rhs_host, "lhs": lhs_host, "srow": srow})
    return in_maps


def combine_outputs(results):
    """results: list of per-core {'losses': [P, MT]} -> scalar loss."""
    rows = []
    for c in range(_NCORES):
        ls = np.asarray(results[c]["losses"])  # [P, MT]
        rows.append(ls.T.reshape(-1))  # row index m*128+p
    all_rows = np.concatenate(rows)
    return np.asarray(all_rows.mean(dtype=np.float64), dtype=np.float32)


def kernel(inputs, targets):
    _import_concourse()
    from concourse.bass_utils import run_bass_kernel_spmd

    nc = build_nc()
    in_maps = prep_inputs(inputs, targets)
    res = run_bass_kernel_spmd(nc, in_maps, core_ids=list(range(_NCORES)))
    return combine_outputs(res.results)


# revision 10
# speedup vs baseline: 1.0721x; 1.0545x over previous
"""Hard-batch-mining triplet loss on 8 Trainium2 NeuronCores.

Reference computation (B=4096, D=2048, NCLS=64, margin=0.3):
    sq = rownorm2(X);  dist = sqrt(clip(sq_i + sq_j - 2 X X^T, 1e-12))
    d_pos = max_{same class} dist;  d_neg = min_{diff class} dist
    loss = mean(relu(d_pos - d_neg + margin))

Distribution: row-parallel. Core c owns 512 rows and computes its
[512, 4096] block of the Gram matrix against all columns.

The whole masked-distance epilogue is folded into the GEMM by augmenting
the contraction dimension:
    rhs  rows: [ X^T (bf16) ; BIG*onehot(class_j) ; hi(sq_j-2048) ; lo ]
    lhsT rows: [ -2*X_blk^T (bf16) ; onehot(class_i) ; 1 ; 1 ]
so PSUM accumulates  p[i,j] = -2*G[i,j] + (sq_j-2048) + BIG*[same class]
in fp32.  Since sqrt/clip are monotone, the masked row reductions reduce
p directly:
    max_j p = BIG + max_same(sq_j - 2048 - 2G)   (BIG dominates)
    min_j p =       min_diff(sq_j - 2048 - 2G)
Per-row epilogue (on [128,1] vectors): add sq_i + 2048 (-BIG), clip,
sqrt, subtract, relu.  Host sums the 4096 per-row losses.
"""

import sys

import numpy as np

_B, _D, _NCLS = 4096, 2048, 64
_NCORES = 8
_MB = _B // _NCORES  # 512 rows per core
_P = 128
_MT = _MB // _P  # 4 m-tiles per core
_NF = 512  # matmul free dim / PSUM bank
_NCH = _B // _NF  # 8 n-chunks
_KT = 17  # 16 k-tiles of X + 1 augmented tile (66 used rows, zero-padded)
_KROWS = _KT * _P
_BIG = 65536.0
_C = 2048.0  # centering constant for sq rows
_MARGIN = 0.3


def _import_concourse():
    try:
        import concourse.bass  # noqa: F401
    except ImportError:
        for p in ("/opt/trn_rl_repo", "/root/.axon_site/_ro/trn_rl_repo"):
            if p not in sys.path:
                sys.path.insert(0, p)
        import concourse.bass  # noqa: F401


def build_nc(finalize=True):
    """Build the single-core Bass/Tile program (SPMD: same program, per-core data)."""
    _import_concourse()
    import concourse.bacc as bacc
    import concourse.mybir as mybir
    import concourse.tile as tile

    f32 = mybir.dt.float32
    bf16 = mybir.dt.bfloat16

    nc = bacc.Bacc(
        "TRN2",
        target_bir_lowering=False,
        debug=False,
        num_devices=_NCORES,
    )

    rhs_d = nc.declare_dram_parameter("rhs", [_NCH, _P, _KT, _NF], bf16, isOutput=False)
    lhs_d = nc.declare_dram_parameter("lhs", [_P, _KT, _MT, _P], bf16, isOutput=False)
    srow_d = nc.declare_dram_parameter("srow", [_P, 2, _MT], f32, isOutput=False)
    loss_d = nc.declare_dram_parameter("losses", [_P, _MT], f32, isOutput=True)

    with tile.TileContext(nc) as tc:
        with (
            tc.tile_pool(name="persist", bufs=1) as persist,
            tc.tile_pool(name="stream", bufs=6) as stream,
            tc.tile_pool(name="ps", bufs=2, space="PSUM") as psp,
            tc.tile_pool(name="small", bufs=1) as small,
        ):
            lhs_t = persist.tile([_P, _KT, _MT, _P], bf16, name="lhs_t")
            srow_t = persist.tile([_P, 2, _MT], f32, name="srow_t")

            maxacc = persist.tile([_P, _MT, _NCH], f32, name="maxacc")
            minacc = persist.tile([_P, _MT, _NCH], f32, name="minacc")
            loss_t = small.tile([_P, _MT], f32, name="loss_t")

            for n in range(_NCH):
                chunk = stream.tile([_P, _KT, _NF], bf16, name="chunk", tag="chunk")
                if n == 0:
                    # Head pieces sized so the first matmuls start early
                    # (each dma_start costs ~600ns of Sync dispatch, so
                    # don't over-split). Issue order alternates lhs/chunk0
                    # to keep the PE's k-order fed.
                    nc.sync.dma_start(out=lhs_t[:, :6], in_=lhs_d[:, :6])
                    nc.sync.dma_start(out=chunk[:, :6], in_=rhs_d[n, :, :6])
                    nc.sync.dma_start(out=lhs_t[:, 6:], in_=lhs_d[:, 6:])
                    nc.sync.dma_start(out=chunk[:, 6:12], in_=rhs_d[n, :, 6:12])
                    nc.sync.dma_start(out=chunk[:, 12:], in_=rhs_d[n, :, 12:])
                else:
                    nc.sync.dma_start(out=chunk[:], in_=rhs_d[n])
                for m in range(_MT):
                    ps = psp.tile([_P, _NF], f32, name="ps", tag=f"ps{m}")
                    for kt in range(_KT):
                        nc.tensor.matmul(
                            ps[:],
                            lhs_t[:, kt, m, :],
                            chunk[:, kt, :],
                            start=(kt == 0),
                            stop=(kt == _KT - 1),
                        )
                    nc.vector.tensor_reduce(
                        maxacc[:, m, n : n + 1],
                        ps[:],
                        axis=mybir.AxisListType.X,
                        op=mybir.AluOpType.max,
                    )
                    nc.vector.tensor_reduce(
                        minacc[:, m, n : n + 1],
                        ps[:],
                        axis=mybir.AxisListType.X,
                        op=mybir.AluOpType.min,
                    )

            # srow is only needed here; keep its DMA off the critical head.
            nc.sync.dma_start(out=srow_t[:], in_=srow_d[:])

            # Vectorized epilogue over all 4 m-tiles at once ([128, 4] ops).
            mx4 = small.tile([_P, _MT], f32, name="mx4")
            mn4 = small.tile([_P, _MT], f32, name="mn4")
            nc.vector.tensor_reduce(
                mx4[:], maxacc[:], axis=mybir.AxisListType.X, op=mybir.AluOpType.max
            )
            nc.vector.tensor_reduce(
                mn4[:], minacc[:], axis=mybir.AxisListType.X, op=mybir.AluOpType.min
            )
            d2p = small.tile([_P, _MT], f32, name="d2p")
            d2n = small.tile([_P, _MT], f32, name="d2n")
            # d2_pos = max(mx + (sq_i + C - BIG), 1e-12); d2_neg likewise.
            nc.vector.tensor_tensor(
                d2p[:], mx4[:], srow_t[:, 0, :], op=mybir.AluOpType.add
            )
            nc.vector.tensor_tensor(
                d2n[:], mn4[:], srow_t[:, 1, :], op=mybir.AluOpType.add
            )
            nc.vector.tensor_scalar_max(d2p[:], d2p[:], 1e-12)
            nc.vector.tensor_scalar_max(d2n[:], d2n[:], 1e-12)
            dp = small.tile([_P, _MT], f32, name="dp")
            dn = small.tile([_P, _MT], f32, name="dn")
            nc.scalar.sqrt(dp[:], d2p[:])
            nc.scalar.sqrt(dn[:], d2n[:])
            diff = small.tile([_P, _MT], f32, name="diff")
            nc.vector.tensor_tensor(
                diff[:], dp[:], dn[:], op=mybir.AluOpType.subtract
            )
            # loss = max(diff + margin, 0)
            nc.vector.tensor_scalar(
                loss_t[:],
                diff[:],
                _MARGIN,
                0.0,
                op0=mybir.AluOpType.add,
                op1=mybir.AluOpType.max,
            )

            nc.sync.dma_start(out=loss_d[:], in_=loss_t[:])

    if finalize:
        nc.finalize()
    return nc


def prep_inputs(inputs, targets):
    """Host-side shard prep. Returns per-core input maps."""
    import ml_dtypes

    X = np.asarray(inputs, dtype=np.float32)
    t = np.asarray(targets).astype(np.int64)
    assert X.shape == (_B, _D) and t.shape == (_B,)

    bf = ml_dtypes.bfloat16
    XTb = np.ascontiguousarray(X.T).astype(bf)  # [D, B] bf16
    sq = np.einsum("ij,ij->i", X.astype(np.float64), X.astype(np.float64)).astype(
        np.float32
    )
    res = sq - np.float32(_C)
    hi = res.astype(bf)
    lo = (res - hi.astype(np.float32)).astype(bf)

    # Shared augmented rhs [KROWS, B] -> DMA layout [NCH, P, KT, NF]
    rhs_aug = np.zeros((_KROWS, _B), dtype=bf)
    rhs_aug[:_D] = XTb
    rhs_aug[_D + t, np.arange(_B)] = bf(_BIG)  # BIG * onehot(class_j)
    rhs_aug[_D + _NCLS] = hi
    rhs_aug[_D + _NCLS + 1] = lo
    rhs_host = np.ascontiguousarray(
        rhs_aug.reshape(_KT, _P, _NCH, _NF).transpose(2, 1, 0, 3)
    )

    in_maps = []
    for c in range(_NCORES):
        blk = slice(c * _MB, (c + 1) * _MB)
        lhs_aug = np.zeros((_KROWS, _MB), dtype=bf)
        lhs_aug[:_D] = (XTb[:, blk].astype(np.float32) * -2.0).astype(bf)  # exact
        tb = t[blk]
        lhs_aug[_D + tb, np.arange(_MB)] = bf(1.0)  # onehot(class_i)
        lhs_aug[_D + _NCLS] = bf(1.0)
        lhs_aug[_D + _NCLS + 1] = bf(1.0)
        lhs_host = np.ascontiguousarray(
            lhs_aug.reshape(_KT, _P, _MT, _P).transpose(1, 0, 2, 3)
        )

        sqb = sq[blk].reshape(_MT, _P)  # [m, p]
        srow = np.empty((_P, 2, _MT), dtype=np.float32)
        srow[:, 0, :] = (sqb + np.float32(_C) - np.float32(_BIG)).T
        srow[:, 1, :] = (sqb + np.float32(_C)).T

        in_maps.append({"rhs": rhs_host, "lhs": lhs_host, "srow": srow})
    return in_maps


def combine_outputs(results):
    """results: list of per-core {'losses': [P, MT]} -> scalar loss."""
    rows = []
    for c in range(_NCORES):
        ls = np.asarray(results[c]["losses"])  # [P, MT]
        rows.append(ls.T.reshape(-1))  # row index m*128+p
    all_rows = np.concatenate(rows)
    return np.asarray(all_rows.mean(dtype=np.float64), dtype=np.float32)


def kernel(inputs, targets):
    _import_concourse()
    from concourse.bass_utils import run_bass_kernel_spmd

    nc = build_nc()
    in_maps = prep_inputs(inputs, targets)
    res = run_bass_kernel_spmd(nc, in_maps, core_ids=list(range(_NCORES)))
    return combine_outputs(res.results)


# revision 12
# speedup vs baseline: 1.0858x; 1.0128x over previous
"""Hard-batch-mining triplet loss on 8 Trainium2 NeuronCores.

Reference computation (B=4096, D=2048, NCLS=64, margin=0.3):
    sq = rownorm2(X);  dist = sqrt(clip(sq_i + sq_j - 2 X X^T, 1e-12))
    d_pos = max_{same class} dist;  d_neg = min_{diff class} dist
    loss = mean(relu(d_pos - d_neg + margin))

Distribution: row-parallel. Core c owns 512 rows and computes its
[512, 4096] block of the Gram matrix against all columns.

The whole masked-distance epilogue is folded into the GEMM by augmenting
the contraction dimension:
    rhs  rows: [ X^T (bf16) ; BIG*onehot(class_j) ; hi(sq_j-2048) ; lo ]
    lhsT rows: [ -2*X_blk^T (bf16) ; onehot(class_i) ; 1 ; 1 ]
so PSUM accumulates  p[i,j] = -2*G[i,j] + (sq_j-2048) + BIG*[same class]
in fp32.  Since sqrt/clip are monotone, the masked row reductions reduce
p directly:
    max_j p = BIG + max_same(sq_j - 2048 - 2G)   (BIG dominates)
    min_j p =       min_diff(sq_j - 2048 - 2G)
Per-row epilogue (on [128,1] vectors): add sq_i + 2048 (-BIG), clip,
sqrt, subtract, relu.  Host sums the 4096 per-row losses.
"""

import sys

import numpy as np

_B, _D, _NCLS = 4096, 2048, 64
_NCORES = 8
_MB = _B // _NCORES  # 512 rows per core
_P = 128
_MT = _MB // _P  # 4 m-tiles per core
_NF = 512  # matmul free dim / PSUM bank
_NCH = _B // _NF  # 8 n-chunks
_KT = 17  # 16 k-tiles of X + 1 augmented tile (66 used rows, zero-padded)
_KROWS = _KT * _P
_BIG = 65536.0
_C = 2048.0  # centering constant for sq rows
_MARGIN = 0.3


def _import_concourse():
    try:
        import concourse.bass  # noqa: F401
    except ImportError:
        for p in ("/opt/trn_rl_repo", "/root/.axon_site/_ro/trn_rl_repo"):
            if p not in sys.path:
                sys.path.insert(0, p)
        import concourse.bass  # noqa: F401


def build_nc(finalize=True):
    """Build the single-core Bass/Tile program (SPMD: same program, per-core data)."""
    _import_concourse()
    import concourse.bacc as bacc
    import concourse.mybir as mybir
    import concourse.tile as tile

    f32 = mybir.dt.float32
    bf16 = mybir.dt.bfloat16

    nc = bacc.Bacc(
        "TRN2",
        target_bir_lowering=False,
        debug=False,
        num_devices=_NCORES,
    )

    rhs_d = nc.declare_dram_parameter("rhs", [_NCH, _P, _KT, _NF], bf16, isOutput=False)
    lhs_d = nc.declare_dram_parameter("lhs", [_P, _KT, _MT, _P], bf16, isOutput=False)
    srow_d = nc.declare_dram_parameter("srow", [_P, 2, _MT], f32, isOutput=False)
    loss_d = nc.declare_dram_parameter("losses", [_P, _MT], f32, isOutput=True)

    with tile.TileContext(nc) as tc:
        with (
            tc.tile_pool(name="persist", bufs=1) as persist,
            tc.tile_pool(name="stream", bufs=6) as stream,
            tc.tile_pool(name="ps", bufs=2, space="PSUM") as psp,
            tc.tile_pool(name="small", bufs=1) as small,
        ):
            # PE warm-up: ~4.5us of junk matmuls on a zeroed scratch tile so
            # the HAM clock-gate opens (K=8/8) during the head DMA wait and
            # the real matmuls start at 2.4 GHz. Borrows one ps slot; the
            # slot is reused (with a sem) long before it's needed again.
            warm = persist.tile([_P, _P], bf16, name="warm")
            nc.gpsimd.memset(warm[:], 0.0)
            ps_warm = psp.tile([_P, _P], f32, name="ps_warm", tag="ps0")
            for _ in range(40):
                nc.tensor.matmul(ps_warm[:], warm[:], warm[:], start=True, stop=True)

            lhs_t = persist.tile([_P, _KT, _MT, _P], bf16, name="lhs_t")
            srow_t = persist.tile([_P, 2, _MT], f32, name="srow_t")

            maxacc = persist.tile([_P, _MT, _NCH], f32, name="maxacc")
            minacc = persist.tile([_P, _MT, _NCH], f32, name="minacc")
            loss_t = small.tile([_P, _MT], f32, name="loss_t")

            for n in range(_NCH):
                chunk = stream.tile([_P, _KT, _NF], bf16, name="chunk", tag="chunk")
                if n == 0:
                    # Head pieces sized so the first matmuls start early
                    # (each dma_start costs ~600ns of Sync dispatch, so
                    # don't over-split). Issue order alternates lhs/chunk0
                    # to keep the PE's k-order fed.
                    nc.sync.dma_start(out=lhs_t[:, :6], in_=lhs_d[:, :6])
                    nc.sync.dma_start(out=chunk[:, :6], in_=rhs_d[n, :, :6])
                    nc.sync.dma_start(out=lhs_t[:, 6:], in_=lhs_d[:, 6:])
                    nc.sync.dma_start(out=chunk[:, 6:12], in_=rhs_d[n, :, 6:12])
                    nc.sync.dma_start(out=chunk[:, 12:], in_=rhs_d[n, :, 12:])
                else:
                    nc.sync.dma_start(out=chunk[:], in_=rhs_d[n])
                for m in range(_MT):
                    ps = psp.tile([_P, _NF], f32, name="ps", tag=f"ps{m}")
                    for kt in range(_KT):
                        nc.tensor.matmul(
                            ps[:],
                            lhs_t[:, kt, m, :],
                            chunk[:, kt, :],
                            start=(kt == 0),
                            stop=(kt == _KT - 1),
                        )
                    nc.vector.tensor_reduce(
                        maxacc[:, m, n : n + 1],
                        ps[:],
                        axis=mybir.AxisListType.X,
                        op=mybir.AluOpType.max,
                    )
                    nc.vector.tensor_reduce(
                        minacc[:, m, n : n + 1],
                        ps[:],
                        axis=mybir.AxisListType.X,
                        op=mybir.AluOpType.min,
                    )

            # srow is only needed here; keep its DMA off the critical head.
            nc.sync.dma_start(out=srow_t[:], in_=srow_d[:])

            # Vectorized epilogue over all 4 m-tiles at once ([128, 4] ops).
            mx4 = small.tile([_P, _MT], f32, name="mx4")
            mn4 = small.tile([_P, _MT], f32, name="mn4")
            nc.vector.tensor_reduce(
                mx4[:], maxacc[:], axis=mybir.AxisListType.X, op=mybir.AluOpType.max
            )
            nc.vector.tensor_reduce(
                mn4[:], minacc[:], axis=mybir.AxisListType.X, op=mybir.AluOpType.min
            )
            d2p = small.tile([_P, _MT], f32, name="d2p")
            d2n = small.tile([_P, _MT], f32, name="d2n")
            # d2_pos = max(mx + (sq_i + C - BIG), 1e-12); d2_neg likewise.
            nc.vector.tensor_tensor(
                d2p[:], mx4[:], srow_t[:, 0, :], op=mybir.AluOpType.add
            )
            nc.vector.tensor_tensor(
                d2n[:], mn4[:], srow_t[:, 1, :], op=mybir.AluOpType.add
            )
            nc.vector.tensor_scalar_max(d2p[:], d2p[:], 1e-12)
            nc.vector.tensor_scalar_max(d2n[:], d2n[:], 1e-12)
            dp = small.tile([_P, _MT], f32, name="dp")
            dn = small.tile([_P, _MT], f32, name="dn")
            nc.scalar.sqrt(dp[:], d2p[:])
            nc.scalar.sqrt(dn[:], d2n[:])
            diff = small.tile([_P, _MT], f32, name="diff")
            nc.vector.tensor_tensor(
                diff[:], dp[:], dn[:], op=mybir.AluOpType.subtract
            )
            # loss = max(diff + margin, 0)
            nc.vector.tensor_scalar(
                loss_t[:],
                diff[:],
                _MARGIN,
                0.0,
                op0=mybir.AluOpType.add,
                op1=mybir.AluOpType.max,
            )

            nc.sync.dma_start(out=loss_d[:], in_=loss_t[:])

    if finalize:
        nc.finalize()
    return nc


def prep_inputs(inputs, targets):
    """Host-side shard prep. Returns per-core input maps."""
    import ml_dtypes

    X = np.asarray(inputs, dtype=np.float32)
    t = np.asarray(targets).astype(np.int64)
    assert X.shape == (_B, _D) and t.shape == (_B,)

    bf = ml_dtypes.bfloat16
    XTb = np.ascontiguousarray(X.T).astype(bf)  # [D, B] bf16
    sq = np.einsum("ij,ij->i", X.astype(np.float64), X.astype(np.float64)).astype(
        np.float32
    )
    res = sq - np.float32(_C)
    hi = res.astype(bf)
    lo = (res - hi.astype(np.float32)).astype(bf)

    # Shared augmented rhs [KROWS, B] -> DMA layout [NCH, P, KT, NF]
    rhs_aug = np.zeros((_KROWS, _B), dtype=bf)
    rhs_aug[:_D] = XTb
    rhs_aug[_D + t, np.arange(_B)] = bf(_BIG)  # BIG * onehot(class_j)
    rhs_aug[_D + _NCLS] = hi
    rhs_aug[_D + _NCLS + 1] = lo
    rhs_host = np.ascontiguousarray(
        rhs_aug.reshape(_KT, _P, _NCH, _NF).transpose(2, 1, 0, 3)
    )

    in_maps = []
    for c in range(_NCORES):
        blk = slice(c * _MB, (c + 1) * _MB)
        lhs_aug = np.zeros((_KROWS, _MB), dtype=bf)
        lhs_aug[:_D] = (XTb[:, blk].astype(np.float32) * -2.0).astype(bf)  # exact
        tb = t[blk]
        lhs_aug[_D + tb, np.arange(_MB)] = bf(1.0)  # onehot(class_i)
        lhs_aug[_D + _NCLS] = bf(1.0)
        lhs_aug[_D + _NCLS + 1] = bf(1.0)
        lhs_host = np.ascontiguousarray(
            lhs_aug.reshape(_KT, _P, _MT, _P).transpose(1, 0, 2, 3)
        )

        sqb = sq[blk].reshape(_MT, _P)  # [m, p]
        srow = np.empty((_P, 2, _MT), dtype=np.float32)
        srow[:, 0, :] = (sqb + np.float32(_C) - np.float32(_BIG)).T
        srow[:, 1, :] = (sqb + np.float32(_C)).T

        in_maps.append({"rhs": rhs_host, "lhs": lhs_host, "srow": srow})
    return in_maps


def combine_outputs(results):
    """results: list of per-core {'losses': [P, MT]} -> scalar loss."""
    rows = []
    for c in range(_NCORES):
        ls = np.asarray(results[c]["losses"])  # [P, MT]
        rows.append(ls.T.reshape(-1))  # row index m*128+p
    all_rows = np.concatenate(rows)
    return np.asarray(all_rows.mean(dtype=np.float64), dtype=np.float32)


def kernel(inputs, targets):
    _import_concourse()
    from concourse.bass_utils import run_bass_kernel_spmd

    nc = build_nc()
    in_maps = prep_inputs(inputs, targets)
    res = run_bass_kernel_spmd(nc, in_maps, core_ids=list(range(_NCORES)))
    return combine_outputs(res.results)
